# revision 30
# baseline (speedup 1.0000x reference)
"""Attention-decoder (B=128, T=256, F=512, O=512, MID=1000, 32 steps) on 8 trn2 cores.

Strategy: data-parallel over batch (16 per core). pre = a @ W1a.T + b1 is
computed once and kept in SBUF as fp16 [MID_p, (b,t)]. Each step:
  u = W1s @ s.T (PE, psum [128m, 16b] x8)          -> tiny
  hpre = pre + u (DVE tensor_scalar, per (mc,b))    -> fp16 4x mode
  h = tanh(hpre) (Act, one [128, 4096] inst per mc) -> act bottleneck
  logitsT[t,b] (PE: stationary=h slices, moving=W2 col, 1-col matmuls)
  softmax over t=partitions: relu+b2 (DVE), exp (Act, f32, no max-sub),
  ssum via ones-stationary matmul, recip (DVE), recip broadcast (PE)
  ctxT[f,b] (PE: stationary=aN tiles, moving=e col), normalize+fp16 (DVE)
  gates gT[o,b] (PE: stationary=WgT, moving=sT/ctxT chunks, bias rank-1)
  LSTM pointwise entirely in transposed [o-part, (oc,b)] layout; s stays
  transposed so the next step needs no transposes at all.
All matmuls keep the big operand stationary (free) and outputs skinny.
"""
import sys
import numpy as np

sys.path.insert(0, "/opt/trn_rl_repo")

DEBUG = False
B, T, F, O, MID = 128, 256, 512, 512, 1000
MIDP = 1024  # padded
NCORES = 8
BC = B // NCORES  # 16 batch per core
BT = BC * T       # 4096


def _build(wo: int):
    import concourse.bass as bass
    import concourse.bacc as bacc
    import concourse.mybir as mybir
    from concourse.tile import TileContext

    f16 = mybir.dt.float16
    f32 = mybir.dt.float32
    AF = mybir.ActivationFunctionType
    OP = mybir.AluOpType

    nc = bacc.Bacc()
    aT_d = nc.dram_tensor("aT", [F, BT], f16, kind="ExternalInput")
    aN_d = nc.dram_tensor("aN", [BT, F], f16, kind="ExternalInput")
    W1aT_d = nc.dram_tensor("W1aT", [F, MIDP], f16, kind="ExternalInput")
    W1sT_d = nc.dram_tensor("W1sT", [O, MIDP], f16, kind="ExternalInput")
    W2c_d = nc.dram_tensor("W2c", [128, 8], f16, kind="ExternalInput")
    b1T_d = nc.dram_tensor("b1T", [128, 8], f32, kind="ExternalInput")
    b2bc_d = nc.dram_tensor("b2bc", [128, 1], f32, kind="ExternalInput")
    WgT_d = nc.dram_tensor("WgT", [O + F, 4 * O], f16, kind="ExternalInput")
    bgT_d = nc.dram_tensor("bgT64", [128, 256], f32, kind="ExternalInput")
    sPT_d = nc.dram_tensor("sPT64", [128, 64], f16, kind="ExternalInput")
    out_d = nc.dram_tensor("out", [wo, 128, 64], f32, kind="ExternalOutput")
    dbg = {}
    if DEBUG:
        dbg["u"] = nc.dram_tensor("dbg_u", [128, 128], f32, kind="ExternalOutput")
        dbg["hpre0"] = nc.dram_tensor("dbg_hpre0", [128, BT], f16, kind="ExternalOutput")
        dbg["h0"] = nc.dram_tensor("dbg_h0", [128, BT], f16, kind="ExternalOutput")
        dbg["lg"] = nc.dram_tensor("dbg_lg", [128, 32], f32, kind="ExternalOutput")
        dbg["e"] = nc.dram_tensor("dbg_e", [128, 32], f32, kind="ExternalOutput")
        dbg["rrow"] = nc.dram_tensor("dbg_rrow", [1, 16], f32, kind="ExternalOutput")
        dbg["alphT"] = nc.dram_tensor("dbg_alphT", [128, 32], f16, kind="ExternalOutput")
        dbg["ctxT"] = nc.dram_tensor("dbg_ctxT", [128, 64], f16, kind="ExternalOutput")
        dbg["cand"] = nc.dram_tensor("dbg_cand", [128, 64], f32, kind="ExternalOutput")
        dbg["sig"] = nc.dram_tensor("dbg_sig", [128, 192], f32, kind="ExternalOutput")
        dbg["pre0"] = nc.dram_tensor("dbg_pre0", [128, BT], f16, kind="ExternalOutput")

    with TileContext(nc) as tc:
        with (
            tc.tile_pool(name="const", bufs=1) as cp,
            tc.tile_pool(name="hpre", bufs=2) as hp,
            tc.tile_pool(name="hh", bufs=2) as hhp,
            tc.tile_pool(name="astream", bufs=2) as app,
            tc.tile_pool(name="strans", bufs=2) as stp,
            tc.tile_pool(name="work", bufs=2) as wp,
            tc.tile_pool(name="small", bufs=2) as sp,
            tc.tile_pool(name="ppre", bufs=2, space="PSUM") as ppre,
            tc.tile_pool(name="psu", bufs=1, space="PSUM") as psup,
            tc.tile_pool(name="plog", bufs=1, space="PSUM") as plogp,
            tc.tile_pool(name="pR", bufs=1, space="PSUM") as pRp,
            tc.tile_pool(name="pctx", bufs=1, space="PSUM") as pctxp,
            tc.tile_pool(name="pgS", bufs=1, space="PSUM") as pgSp,
            tc.tile_pool(name="pgC", bufs=1, space="PSUM") as pgCp,
        ):
            dma = nc.sync.dma_start

            # ---- constant loads ----
            aN_sb = {}
            for b in range(BC):
                for tcn in range(2):
                    t_ = cp.tile([128, F], f16, tag=f"aN{b}_{tcn}", name=f"aN{b}_{tcn}")
                    dma(t_[:], aN_d[b * T + tcn * 128: b * T + (tcn + 1) * 128, :])
                    aN_sb[(b, tcn)] = t_
            W1aT_sb, W1sT_sb, WgT_sb = [], [], []
            for kc in range(4):
                t_ = cp.tile([128, MIDP], f16, tag=f"w1a{kc}", name=f"w1a{kc}")
                dma(t_[:], W1aT_d[kc * 128:(kc + 1) * 128, :])
                W1aT_sb.append(t_)
            for kc in range(4):
                t_ = cp.tile([128, MIDP], f16, tag=f"w1s{kc}", name=f"w1s{kc}")
                dma(t_[:], W1sT_d[kc * 128:(kc + 1) * 128, :])
                W1sT_sb.append(t_)
            for kc in range(8):
                t_ = cp.tile([128, 4 * O], f16, tag=f"wg{kc}", name=f"wg{kc}")
                dma(t_[:], WgT_d[kc * 128:(kc + 1) * 128, :])
                WgT_sb.append(t_)
            W2_sb = cp.tile([128, 8], f16, tag="w2", name="w2")
            dma(W2_sb[:], W2c_d[:])
            b1T_sb = cp.tile([128, 8], f32, tag="b1t", name="b1t")
            dma(b1T_sb[:], b1T_d[:])
            b2bc_sb = cp.tile([128, 1], f32, tag="b2bc", name="b2bc")
            dma(b2bc_sb[:], b2bc_d[:])
            bgT_sb = cp.tile([128, 256], f32, tag="bgT", name="bgT")
            dma(bgT_sb[:], bgT_d[:])
            onesf = cp.tile([128, 128], f32, tag="onesf", name="onesf")
            nc.vector.memset(onesf[:], 1.0)

            sT = stp.tile([128, 64], f16, tag="sT", name="sT0")
            dma(sT[:], sPT_d[:])
            cT = wp.tile([128, 64], f32, tag="cT", name="cT0")
            nc.vector.memset(cT[:], 0.0)

            # ---- precompute pre = (a @ W1a.T).T + b1 : [MID_p, (b,t)] fp16 ----
            pre_sb = []
            for mc in range(8):
                pre_sb.append(cp.tile([128, BT], f16, tag=f"pre{mc}", name=f"pre{mc}"))
            for ns in range(8):
                a_sl = []
                for kc in range(4):
                    t_ = app.tile([128, 512], f16, tag=f"astr{kc}", name=f"astr{kc}")
                    dma(t_[:], aT_d[kc * 128:(kc + 1) * 128, ns * 512:(ns + 1) * 512])
                    a_sl.append(t_)
                for mc in range(8):
                    ps = ppre.tile([128, 512], f32, tag="ppre", name="ppre")
                    for kc in range(4):
                        nc.tensor.matmul(
                            ps[:],
                            W1aT_sb[kc][:, mc * 128:(mc + 1) * 128],
                            a_sl[kc][:],
                            start=(kc == 0), stop=(kc == 3),
                        )
                    dst = pre_sb[mc][:, ns * 512:(ns + 1) * 512]
                    if mc % 2 == 0:
                        nc.scalar.activation(dst, ps[:], AF.Identity,
                                             bias=b1T_sb[:, mc:mc + 1], scale=1.0)
                    else:
                        nc.vector.tensor_scalar(
                            out=dst, in0=ps[:], scalar1=b1T_sb[:, mc:mc + 1],
                            scalar2=None, op0=OP.add)

            # ---- decode steps ----
            for t in range(wo):
                # u.T = W1s @ s.T : psum [128, (mc,b)] f32
                psu = psup.tile([128, 128], f32, tag="psu", name="psu")
                for mc in range(8):
                    for kc in range(4):
                        nc.tensor.matmul(
                            psu[:, mc * 16:(mc + 1) * 16],
                            W1sT_sb[kc][:, mc * 128:(mc + 1) * 128],
                            sT[:, kc * 16:(kc + 1) * 16],
                            start=(kc == 0), stop=(kc == 3),
                        )

                # gates s-half early (overlaps the tanh chain):
                # pgS [128, (g,oc,b)] = Wg_s.T s ; bias added on the copy out
                pgS = pgSp.tile([128, 256], f32, tag="pgS", name="pgS")
                for g in range(4):
                    for oc in range(4):
                        col = g * 64 + oc * 16
                        for kc in range(4):
                            nc.tensor.matmul(
                                pgS[:, col:col + 16],
                                WgT_sb[kc][:, g * O + oc * 128: g * O + (oc + 1) * 128],
                                sT[:, kc * 16:(kc + 1) * 16],
                                start=(kc == 0), stop=(kc == 3),
                            )
                # park s-half + bias in SBUF (overlaps the tanh chain) so the
                # tail add reads only one PSUM operand
                gS = sp.tile([128, 256], f32, tag="gS", name="gS")
                nc.vector.tensor_tensor(out=gS[:], in0=pgS[:], in1=bgT_sb[:],
                                        op=OP.add)

                # hpre = pre + u (DVE, fp16 4x), h = tanh(hpre) (Act), then
                # logits partials (PE, stationary=h slices; col = mc*32+tc*16+b,
                # every matmul its own closed psum group)
                plog = plogp.tile([128, 512], f32, tag="plog", name="plog")
                for mc in range(8):
                    hpre = hp.tile([128, BT], f16, tag="hpre", name="hpre")
                    for b in range(BC):
                        nc.vector.tensor_scalar(
                            out=hpre[:, b * T:(b + 1) * T],
                            in0=pre_sb[mc][:, b * T:(b + 1) * T],
                            scalar1=psu[:, mc * 16 + b: mc * 16 + b + 1],
                            scalar2=None, op0=OP.add,
                        )
                    h = hhp.tile([128, BT], f16, tag="h", name="h")
                    nc.scalar.activation(h[:], hpre[:], AF.Tanh)
                    if DEBUG and t == 0 and mc == 0:
                        dma(dbg["pre0"][:], pre_sb[0][:])
                        dma(dbg["hpre0"][:], hpre[:])
                        dma(dbg["h0"][:], h[:])
                    for b in range(BC):
                        for tcn in range(2):
                            col = mc * 32 + tcn * 16 + b
                            nc.tensor.matmul(
                                plog[:, col: col + 1],
                                h[:, b * T + tcn * 128: b * T + (tcn + 1) * 128],
                                W2_sb[:, mc:mc + 1],
                                start=True, stop=True,
                            )

                # reduce the 8 mc partials: copy to SBUF, pairwise add tree
                lcp = sp.tile([128, 256], f32, tag="lcp", name="lcp")
                nc.vector.tensor_copy(lcp[:], plog[:, 0:256])
                ra = sp.tile([128, 128], f32, tag="ra", name="ra")
                nc.vector.tensor_tensor(out=ra[:], in0=lcp[:, 0:128],
                                        in1=lcp[:, 128:256], op=OP.add)
                rb = sp.tile([128, 64], f32, tag="rb", name="rb")
                nc.vector.tensor_tensor(out=rb[:], in0=ra[:, 0:64],
                                        in1=ra[:, 64:128], op=OP.add)
                rc = sp.tile([128, 32], f32, tag="rc", name="rc")
                nc.vector.tensor_tensor(out=rc[:], in0=rb[:, 0:32],
                                        in1=rb[:, 32:64], op=OP.add)
                lg = sp.tile([128, 32], f32, tag="lg", name="lg")
                nc.vector.tensor_scalar(
                    out=lg[:], in0=rc[:], scalar1=b2bc_sb[:, 0:1],
                    scalar2=0.0, op0=OP.add, op1=OP.max,
                )
                e = sp.tile([128, 32], f32, tag="e", name="e")
                nc.scalar.activation(e[:], lg[:], AF.Exp)
                srow = plog[0:1, 256:272]
                for tcn in range(2):
                    nc.tensor.matmul(
                        srow, onesf[:, 0:1], e[:, tcn * 16:(tcn + 1) * 16],
                        start=(tcn == 0), stop=(tcn == 1),
                    )
                rrow = sp.tile([1, 16], f32, tag="rrow", name="rrow")
                nc.vector.reciprocal(rrow[:], srow[:])
                # broadcast recip to [128, (tc,b)] psum, then alphaT fp16
                pR = pRp.tile([128, 32], f32, tag="pR", name="pR")
                for tcn in range(2):
                    nc.tensor.matmul(pR[:, tcn * 16:(tcn + 1) * 16],
                                     onesf[0:1, :], rrow[:],
                                     start=True, stop=True)
                alphT = sp.tile([128, 32], f16, tag="alphT", name="alphT")
                nc.vector.tensor_tensor(out=alphT[:], in0=e[:], in1=pR[:],
                                        op=OP.mult)

                # ctxT[f, b] = sum_t aN[t, f] * alpha[t, b]
                pctx = pctxp.tile([128, 64], f32, tag="pctx", name="pctx")
                for b in range(BC):
                    for fc in range(4):
                        for tcn in range(2):
                            nc.tensor.matmul(
                                pctx[:, fc * 16 + b: fc * 16 + b + 1],
                                aN_sb[(b, tcn)][:, fc * 128:(fc + 1) * 128],
                                alphT[:, tcn * 16 + b: tcn * 16 + b + 1],
                                start=(tcn == 0), stop=(tcn == 1),
                            )
                ctxT = stp.tile([128, 64], f16, tag="ctxT", name="ctxT")
                nc.vector.tensor_copy(ctxT[:], pctx[:])

                # gates ctx-half
                pgC = pgCp.tile([128, 256], f32, tag="pgC", name="pgC")
                for g in range(4):
                    for oc in range(4):
                        col = g * 64 + oc * 16
                        for kc in range(4, 8):
                            nc.tensor.matmul(
                                pgC[:, col:col + 16],
                                WgT_sb[kc][:, g * O + oc * 128: g * O + (oc + 1) * 128],
                                ctxT[:, (kc - 4) * 16:(kc - 3) * 16],
                                start=(kc == 4), stop=(kc == 7),
                            )
                gsum = sp.tile([128, 256], f32, tag="gsum", name="gsum")
                nc.vector.tensor_tensor(out=gsum[:], in0=pgC[:], in1=gS[:],
                                        op=OP.add)

                # gate activations: cand = tanh(g0); others = 0.5*tanh(0.5x)+0.5
                cand = sp.tile([128, 64], f32, tag="cand", name="cand")
                nc.scalar.activation(cand[:], gsum[:, 0:64], AF.Tanh)
                tT = sp.tile([128, 192], f32, tag="tT", name="tT")
                nc.scalar.activation(tT[:], gsum[:, 64:256], AF.Tanh, scale=0.5)
                sig = sp.tile([128, 192], f32, tag="sig", name="sig")
                nc.vector.tensor_scalar(out=sig[:], in0=tT[:], scalar1=0.5,
                                        scalar2=0.5, op0=OP.mult, op1=OP.add)

                if DEBUG and t == 0:
                    ucp = sp.tile([128, 128], f32, tag="ucp", name="ucp")
                    nc.vector.tensor_copy(ucp[:], psu[:])
                    dma(dbg["u"][:], ucp[:])
                    dma(dbg["lg"][:], lg[:])
                    dma(dbg["e"][:], e[:])
                    dma(dbg["rrow"][:], rrow[:])
                    dma(dbg["alphT"][:], alphT[:])
                    dma(dbg["ctxT"][:], ctxT[:])
                    dma(dbg["cand"][:], cand[:])
                    dma(dbg["sig"][:], sig[:])

                # LSTM pointwise in transposed layout
                t1 = sp.tile([128, 64], f32, tag="t1", name="t1")
                nc.vector.tensor_tensor(out=t1[:], in0=sig[:, 0:64], in1=cand[:],
                                        op=OP.mult)
                t2 = sp.tile([128, 64], f32, tag="t2", name="t2")
                nc.gpsimd.tensor_tensor(out=t2[:], in0=sig[:, 64:128], in1=cT[:],
                                        op=OP.mult)
                cT_new = wp.tile([128, 64], f32, tag="cT", name="cT")
                nc.vector.tensor_tensor(out=cT_new[:], in0=t1[:], in1=t2[:],
                                        op=OP.add)
                tch = sp.tile([128, 64], f32, tag="tch", name="tch")
                nc.scalar.activation(tch[:], cT_new[:], AF.Tanh)
                sOut = wp.tile([128, 64], f32, tag="sOut", name="sOut")
                nc.vector.tensor_tensor(out=sOut[:], in0=sig[:, 128:192],
                                        in1=tch[:], op=OP.mult)
                dma(out_d[t, :, :], sOut[:])
                cT = cT_new
                if t + 1 < wo:
                    sT_new = stp.tile([128, 64], f16, tag="sT", name="sT")
                    nc.vector.tensor_copy(sT_new[:], sOut[:])
                    sT = sT_new
    nc.compile()
    return nc


def _make_runner(nc):
    """Build the sharded jit callable ONCE per module (run_bass_via_pjrt
    rebuilds it per call, costing seconds of retrace/recompile)."""
    import jax
    import numpy as _np
    from jax.sharding import Mesh, PartitionSpec
    from jax.experimental.shard_map import shard_map
    from concourse import bass2jax, mybir

    bass2jax.install_neuronx_cc_hook()
    partition_name = nc.partition_id_tensor.name if nc.partition_id_tensor else None
    in_names, out_names, out_avals, zero_outs = [], [], [], []
    for alloc in nc.m.functions[0].allocations:
        if not isinstance(alloc, mybir.MemoryLocationSet):
            continue
        name = alloc.memorylocations[0].name
        if alloc.kind == "ExternalInput":
            if name != partition_name:
                in_names.append(name)
        elif alloc.kind == "ExternalOutput":
            shape = tuple(alloc.tensor_shape)
            dtype = mybir.dt.np(alloc.dtype)
            out_names.append(name)
            out_avals.append(jax.core.ShapedArray(shape, dtype))
            zero_outs.append(_np.zeros(shape, dtype))
    n_params = len(in_names)
    n_outs = len(out_avals)
    in_names_all = list(in_names) + list(out_names)
    if partition_name is not None:
        in_names_all.append(partition_name)

    def _body(*args):
        operands = list(args)
        if partition_name is not None:
            operands.append(bass2jax.partition_id_tensor())
        outs = bass2jax._bass_exec_p.bind(
            *operands,
            out_avals=tuple(out_avals),
            in_names=tuple(in_names_all),
            out_names=tuple(out_names),
            lowering_input_output_aliases=(),
            sim_require_finite=True,
            sim_require_nnan=True,
            nc=nc,
        )
        return tuple(outs)

    donate = tuple(range(n_params, n_params + n_outs))
    devices = jax.devices()[:NCORES]
    mesh = Mesh(_np.asarray(devices), ("core",))
    sharded = jax.jit(
        shard_map(_body, mesh=mesh,
                  in_specs=(PartitionSpec("core"),) * (n_params + n_outs),
                  out_specs=(PartitionSpec("core"),) * n_outs,
                  check_rep=False),
        donate_argnums=donate, keep_unused=True,
    )

    def run(in_maps):
        concat_in = [
            np.concatenate([np.asarray(in_maps[c][nm]) for c in range(NCORES)], axis=0)
            for nm in in_names[:n_params]
        ]
        concat_zeros = [np.zeros((NCORES * z.shape[0], *z.shape[1:]), z.dtype)
                        for z in zero_outs]
        out_arrs = sharded(*concat_in, *concat_zeros)
        return [
            {nm: np.asarray(out_arrs[i]).reshape(NCORES, *out_avals[i].shape)[c]
             for i, nm in enumerate(out_names)}
            for c in range(NCORES)
        ]

    run.sharded = sharded
    run.zero_outs = zero_outs
    run.in_names = in_names[:n_params]
    run.out_names = out_names
    run.out_avals = out_avals
    return run


_BUILT = {}


def kernel(**inputs):
    a = np.asarray(inputs["a"], np.float32)
    s_prev = np.asarray(inputs["s_prev"], np.float32)
    W1 = np.asarray(inputs["W1"], np.float32)
    b1 = np.asarray(inputs["b1"], np.float32)
    W2 = np.asarray(inputs["W2"], np.float32)
    b2 = np.asarray(inputs["b2"], np.float32)
    w_c = np.asarray(inputs["w_c"], np.float32)
    w_u = np.asarray(inputs["w_u"], np.float32)
    w_f = np.asarray(inputs["w_f"], np.float32)
    w_o = np.asarray(inputs["w_o"], np.float32)
    b_c = np.asarray(inputs["b_c"], np.float32)
    b_u = np.asarray(inputs["b_u"], np.float32)
    b_f = np.asarray(inputs["b_f"], np.float32)
    b_o = np.asarray(inputs["b_o"], np.float32)
    wo = int(np.asarray(inputs["word_output"]))

    if wo not in _BUILT:
        nc_ = _build(wo)
        _BUILT[wo] = (nc_, _make_runner(nc_))
    nc, runner = _BUILT[wo]

    W1aT = np.zeros((F, MIDP), np.float16)
    W1aT[:, :MID] = W1[:, :F].T
    W1sT = np.zeros((O, MIDP), np.float16)
    W1sT[:, :MID] = W1[:, F:].T
    W2p = np.zeros((MIDP,), np.float32)
    W2p[:MID] = W2[0]
    W2c = W2p.reshape(8, 128).T.astype(np.float16)
    b1p = np.zeros((MIDP,), np.float32)
    b1p[:MID] = b1
    b1T = b1p.reshape(8, 128).T.copy()
    WgT = np.concatenate([w.T for w in (w_c, w_u, w_f, w_o)], axis=1).astype(np.float16)
    bgv = np.concatenate([b_c, b_u, b_f, b_o]).astype(np.float32)  # [2048]
    # bgT64[p, g*64+oc*16+b] = bg[g*512+oc*128+p]
    bgT64 = np.ascontiguousarray(
        bgv.reshape(4, 4, 128).transpose(2, 0, 1)[:, :, :, None]
        .repeat(BC, 3).reshape(128, 256))
    common = {
        "W1aT": W1aT, "W1sT": W1sT, "W2c": W2c, "b1T": b1T,
        "b2bc": np.full((128, 1), float(b2.reshape(-1)[0]), np.float32),
        "WgT": WgT, "bgT64": bgT64,
    }
    in_maps = []
    for c in range(NCORES):
        b0 = c * BC
        ac = a[b0:b0 + BC]
        sc = s_prev[b0:b0 + BC]  # [16, 512]
        sT64 = sc.T.reshape(4, 128, BC).transpose(1, 0, 2).reshape(128, 64)
        in_maps.append({
            **common,
            "aT": np.ascontiguousarray(ac.transpose(2, 0, 1).reshape(F, BT)).astype(np.float16),
            "aN": np.ascontiguousarray(ac.reshape(BT, F)).astype(np.float16),
            "sPT64": np.ascontiguousarray(sT64).astype(np.float16),
        })

    results = None
    for attempt in range(4):
        try:
            results = runner(in_maps)
            break
        except Exception:
            if attempt == 3:
                raise
            import time as _time
            _time.sleep(1.0)
            if attempt >= 1:
                runner = _make_runner(nc)
                _BUILT[wo] = (nc, runner)
    global _LAST_RESULTS
    _LAST_RESULTS = results
    out = np.empty((B, wo, O), np.float32)
    for c in range(NCORES):
        arr = results[c]["out"]  # [wo, 128, 64]
        out[c * BC:(c + 1) * BC] = (
            arr.reshape(wo, 128, 4, BC).transpose(3, 0, 2, 1).reshape(BC, wo, O)
        )
    return out
